# revision 61
# baseline (speedup 1.0000x reference)
"""Trainium2 Bass kernel for nn_DecoderLayer (dense transformer decoder layer).

Sharding: data-parallel over batch (16 batches -> 8 cores x 2 each). Each core
runs the full decoder layer on its batch slice; no collectives.

v7: all weight/input layout transformation happens host-side in kernel()
(numpy): weights are pre-transposed to W^T block-major bf16, enc_mem / query /
pos-enc are pre-transposed to feature-major bf16, and q+pos_enc is
pre-summed. The device program is then pure GEMM streaming: the SP queue
carries only plain contiguous DMAs, ACT only exp/LN/relu, DVE only PSUM
evictions and element-wise adds. This avoids the on-chip DMA->cast->XBAR
producer chains whose in-order queue waits kept head-of-line-blocking the
enc-chunk loads and starving the PE (HAM then re-throttles it to 1.2 GHz,
doubling every matmul).

Layout: activations are feature-major ("xT": [feature partitions, token
free]) so every linear is psum = W^T.T @ xT with bf16 operands and fp32 PSUM
accumulation. Attention uses transposed scores S^T = K_hT.T @ Q_hT
([j partitions, i free]), exp without max-subtraction (scores are bounded),
and a ones-column appended to V so the softmax denominator comes out of the
same PV matmul. LayerNorm runs feature-major with partition sums via
ones-vector matmuls and per-token broadcast via K=1 matmuls.

The cross-attention main loop is software-pipelined: chunk c's K/V-projection
matmul groups interleave with chunk c-1's attention at head-pair granularity
(scores and PV split one slot apart), so the PE always has projection work
while ACT grinds the exps and HAM stays at full clock.
"""
import sys
import numpy as np

sys.path.insert(0, '/opt/trn_rl_repo')

import ml_dtypes  # noqa: E402
import concourse.bass as bass  # noqa: E402
import concourse.tile as tile  # noqa: E402
from concourse import bacc, mybir  # noqa: E402
from concourse.bass_utils import run_bass_kernel_spmd  # noqa: E402
from concourse.masks import make_identity  # noqa: E402
from contextlib import ExitStack  # noqa: E402

F32 = mybir.dt.float32
BF16 = mybir.dt.bfloat16
AF = mybir.ActivationFunctionType
BF16_NP = ml_dtypes.bfloat16

EPS = 1e-5
N_CORES = 8


def build_decoder(nc, tc, ctx, B_loc, NQ, S, W, NH, MLP, JC=512, suffix=""):
    HD = W // NH
    assert HD == 64 and NQ % 128 == 0 and W % 512 == 0 and JC % 128 == 0
    T = B_loc * NQ          # decoder tokens per core
    TC = T // 128
    WC = W // 128
    MC = MLP // 128
    NJC = S // JC           # enc chunks per batch
    JSC = JC // 128
    NQC = NQ // 128
    SCALE = float(W) ** -0.5
    HPC = 128 // HD         # heads per feature chunk (2)

    # Transformed inputs (host-side marshalling):
    #   *_wt:  [128, O/128, I/128, 128] bf16, wt[p,ob,c,n] = W[ob*128+n, c*128+p]
    #   qT/qkT/peT: [128, WC, T] bf16 feature-major (qkT = q + pos_enc)
    #   mT:    [128, WC, T_enc] bf16 feature-major (T_enc = B_loc*S)
    #   *_col: [128, n/128] f32 per-partition param columns
    dram = {}
    for name, shape, dt in (
        [('qT', [128, WC, T], BF16), ('qkT', [128, WC, T], BF16),
         ('peT', [128, WC, T], BF16), ('mT', [128, WC, B_loc * S], BF16),
         ('ffn_w1t', [128, MC, WC, 128], BF16),
         ('ffn_w2t', [128, WC, MC, 128], BF16),
         ('b1_col', [128, MC], F32), ('b2_col', [128, WC], F32)] +
        [(f'{p}_w{k}t', [128, WC, WC, 128], BF16)
         for p in ('sa', 'ca') for k in 'qkvo'] +
        [(f'ln{i}_{gb}_col', [128, WC], F32)
         for i in (1, 2, 3) for gb in ('g', 'b')]
    ):
        if suffix:
            dram[name] = build_decoder._dram_cache[name]
        else:
            dram[name] = nc.dram_tensor(name, shape, dt, kind="ExternalInput")
    build_decoder._dram_cache = dict(dram)
    out_d = nc.dram_tensor("out" + suffix, [B_loc, NQ, W], BF16,
                           kind="ExternalOutput")
    out_flat = out_d.rearrange("b n w -> (b n) w")

    # ---------------- global pools ----------------
    consts = ctx.enter_context(tc.tile_pool(name="consts", bufs=1))
    persist = ctx.enter_context(tc.tile_pool(name="persist", bufs=1))
    scratch = ctx.enter_context(tc.tile_pool(name="scratch", bufs=2))
    mm_ps = ctx.enter_context(tc.tile_pool(name="mm_ps", bufs=3, space="PSUM"))
    sc_ps = ctx.enter_context(tc.tile_pool(name="sc_ps", bufs=2, space="PSUM"))
    pv_ps = ctx.enter_context(tc.tile_pool(name="pv_ps", bufs=3, space="PSUM"))

    ident = consts.tile([128, 128], F32, tag="ident")
    make_identity(nc, ident[:])
    ones_f = consts.tile([128, 128], F32, tag="ones_f")
    nc.gpsimd.memset(ones_f[:], 1.0)
    ones_b = consts.tile([128, 128], BF16, tag="ones_b")
    nc.vector.tensor_copy(ones_b[:], ones_f[:])
    eps_t = consts.tile([1, 1], F32, tag="eps")
    nc.gpsimd.memset(eps_t[:], EPS)

    cols = {}
    for name in ['ln1_g', 'ln1_b', 'ln2_g', 'ln2_b', 'ln3_g', 'ln3_b',
                 'b2']:
        cn = name + '_col' if name != 'b2' else 'b2_col'
        t = consts.tile([128, WC], F32, tag=cn, name=cn)
        nc.sync.dma_start(t[:], dram[cn][:, :])
        cols[name] = t
    b1_col = consts.tile([128, MC], F32, tag="b1c", name="b1_col")
    nc.sync.dma_start(b1_col[:], dram['b1_col'][:, :])

    # ---------------- helpers ----------------
    def load_wT(pool, tag, wt_d, name, bufs=1):
        """bf16 W^T tile from pre-transposed DRAM; per-ob DMAs so the first
        GEMM group can start after one 256KB transfer."""
        OB = wt_d.shape[1]
        wt = pool.tile([128, OB, wt_d.shape[2], 128], BF16, tag=tag,
                       name=name, bufs=bufs)
        for ob in range(OB):
            nc.sync.dma_start(wt[:, ob], wt_d[:, ob])
        return wt

    def gemm(psum, wt, ob, rhs, ICn):
        """psum[oc 128, N] += sum_ic wt[:, ob, ic, :].T @ rhs(ic)."""
        for ic in range(ICn):
            nc.tensor.matmul(psum, wt[:, ob, ic, :], rhs(ic),
                             start=(ic == 0), stop=(ic == ICn - 1))

    def layernorm(x_fn, n_chunks, N, g_col, b_col, out_fn):
        """Feature-major LN over the partition (feature) dim."""
        ps_s = sc_ps.tile([1, N], F32, tag="sc", name="ps_s",
                          padded_shape=[1, 512])
        for ic in range(n_chunks):
            nc.tensor.matmul(ps_s[0:1, :], ones_b[:, 0:1], x_fn(ic),
                             start=(ic == 0), stop=(ic == n_chunks - 1))
        ps_q = sc_ps.tile([1, N], F32, tag="sc", name="ps_q",
                          padded_shape=[1, 512])
        for ic in range(n_chunks):
            sq = scratch.tile([128, N], BF16, tag="sq", name="sq",
                              padded_shape=[128, 512])
            nc.vector.tensor_mul(sq[:, 0:N], x_fn(ic), x_fn(ic))
            nc.tensor.matmul(ps_q[0:1, :], ones_b[:, 0:1], sq[:, 0:N],
                             start=(ic == 0), stop=(ic == n_chunks - 1))
        inv_w = 1.0 / (n_chunks * 128)
        mu = scratch.tile([1, N], BF16, tag="st_mu", bufs=1, name="mu")
        nc.scalar.activation(mu[0:1, :], ps_s[0:1, :], AF.Copy, scale=inv_w)
        ex2 = scratch.tile([1, N], F32, tag="st_e", bufs=1, name="ex2")
        nc.scalar.activation(ex2[0:1, :], ps_q[0:1, :], AF.Copy, scale=inv_w)
        mu2 = scratch.tile([1, N], F32, tag="st_x", bufs=1, name="mu2")
        nc.vector.tensor_mul(mu2[0:1, :], mu[0:1, :], mu[0:1, :])
        var = scratch.tile([1, N], F32, tag="st_v", bufs=1, name="var")
        nc.vector.tensor_sub(var[0:1, :], ex2[0:1, :], mu2[0:1, :])
        sd = scratch.tile([1, N], F32, tag="st_x", bufs=1, name="sd")
        nc.scalar.activation(sd[0:1, :], var[0:1, :], AF.Sqrt,
                             bias=eps_t[0:1, 0:1])
        rstd = scratch.tile([1, N], BF16, tag="st_r", bufs=1, name="rstd")
        nc.vector.reciprocal(rstd[0:1, :], sd[0:1, :])
        ps_mu = sc_ps.tile([128, N], F32, tag="sc", name="ps_mu",
                           padded_shape=[128, 512])
        nc.tensor.matmul(ps_mu[:, 0:N], ones_b[0:1, :], mu[0:1, :])
        ps_rs = sc_ps.tile([128, N], F32, tag="sc", name="ps_rs",
                           padded_shape=[128, 512])
        nc.tensor.matmul(ps_rs[:, 0:N], ones_b[0:1, :], rstd[0:1, :])
        for ic in range(n_chunks):
            xm = scratch.tile([128, N], F32, tag="xm", bufs=1, name="xm",
                              padded_shape=[128, 512])
            nc.vector.tensor_sub(xm[:, 0:N], x_fn(ic), ps_mu[:, 0:N])
            nc.vector.tensor_mul(xm[:, 0:N], xm[:, 0:N], ps_rs[:, 0:N])
            nc.scalar.activation(out_fn(ic), xm[:, 0:N], AF.Identity,
                                 bias=b_col[:, ic:ic + 1],
                                 scale=g_col[:, ic:ic + 1])

    def normalize_head(h, src, oT, col):
        """oT head slice = src[0:HD] / src[HD] (softmax sums row)."""
        off = (h % HPC) * HD
        fc = h // HPC
        rec = scratch.tile([1, NQ], BF16, tag="rec", bufs=2, name="rec")
        nc.vector.reciprocal(rec[0:1, :], src[HD:HD + 1, :])
        ps_b = mm_ps.tile([HD, NQ], F32, tag="mm", name="ps_bc")
        nc.tensor.matmul(ps_b[0:HD, :], ones_b[0:1, 0:HD], rec[0:1, :])
        nc.vector.tensor_mul(oT[off:off + HD, fc, col:col + NQ], src[0:HD, :],
                             ps_b[0:HD, :])

    def sa_scores_pair(b, ksaT, qsaT, hp):
        """Scores + exp for head pair hp of batch b (SA, NQC key blocks)."""
        es = []
        for sub in range(2):
            h = 2 * hp + sub
            off = (h % HPC) * HD
            fc = h // HPC
            e = sa_pool[0].tile([128, NQC, NQ], BF16, tag="expsa",
                                bufs=16, name="esa")
            ps_s = sc_ps.tile([128, 2, NQ], F32, tag="sc", name="ps_sc")
            for js in range(NQC):
                nc.tensor.matmul(
                    ps_s[:, js, :],
                    ksaT[off:off + HD, fc, b * NQ + js * 128:
                         b * NQ + (js + 1) * 128],
                    qsaT[off:off + HD, fc, b * NQ:(b + 1) * NQ])
            nc.scalar.activation(e[:, :, :], ps_s[:, 0:NQC, :], AF.Exp,
                                 scale=SCALE)
            es.append(e)
        return es

    def sa_pv_pair(b, vext_all, hp, es, oT):
        ps_o = pv_ps.tile([HD + 1, 2, NQ], F32, tag="pv", name="ps_pv2")
        for sub, e in ((0, es[0]), (1, es[1])):
            h = 2 * hp + sub
            for js in range(NQC):
                nc.tensor.matmul(
                    ps_o[0:HD + 1, sub, :],
                    vext_all[:, b * NQC + js, h, :], e[:, js, :],
                    start=(js == 0), stop=(js == NQC - 1))
        pv_sb = sa_pool[0].tile([HD + 1, 2, NQ], F32, tag="pvsb", bufs=2,
                                name="pv_sb")
        nc.vector.tensor_copy(pv_sb[:], ps_o[:])
        normalize_head(2 * hp, pv_sb[:, 0, :], oT, b * NQ)
        normalize_head(2 * hp + 1, pv_sb[:, 1, :], oT, b * NQ)

    # ================= P0: self-attention =================
    sa_pool = [None]
    x1T = persist.tile([128, WC, T], BF16, tag="x1T", name="x1T")
    qT = persist.tile([128, WC, T], BF16, tag="qT", name="qT")
    peT = persist.tile([128, WC, T], BF16, tag="peT", name="peT")
    with nc.named_scope("sa"), \
         tc.tile_pool(name="sa_w", bufs=1) as sa_w, \
         tc.tile_pool(name="sa", bufs=1) as sa:
        sa_pool[0] = sa
        qkT = sa.tile([128, WC, T], BF16, tag="qkT", name="qkT")
        nc.sync.dma_start(qkT[:], dram['qkT'][:, :, :])
        nc.sync.dma_start(qT[:], dram['qT'][:, :, :])
        nc.sync.dma_start(peT[:], dram['peT'][:, :, :])

        wqt = load_wT(sa_w, "wtA", dram['sa_wqt'], "sa_wq_t", bufs=2)
        qsaT = sa.tile([128, WC, T], BF16, tag="big", bufs=3, name="qsaT")
        for ob in range(WC):
            ps = mm_ps.tile([128, T], F32, tag="mm", name="ps_q")
            gemm(ps[:, 0:T], wqt, ob, lambda ic: qkT[:, ic, :], WC)
            nc.vector.tensor_copy(qsaT[:, ob, :], ps[:, 0:T])
        wkt = load_wT(sa_w, "wtA", dram['sa_wkt'], "sa_wk_t", bufs=2)
        ksaT = sa.tile([128, WC, T], BF16, tag="big", bufs=3, name="ksaT")
        for ob in range(WC):
            ps = mm_ps.tile([128, T], F32, tag="mm", name="ps_k")
            gemm(ps[:, 0:T], wkt, ob, lambda ic: qkT[:, ic, :], WC)
            nc.vector.tensor_copy(ksaT[:, ob, :], ps[:, 0:T])
        wvt = load_wT(sa_w, "wtA", dram['sa_wvt'], "sa_wv_t", bufs=2)
        wot = load_wT(sa_w, "wtA", dram['sa_wot'], "sa_wo_t", bufs=2)
        vext_all = sa.tile([128, TC, NH, HD + 1], BF16, tag="vext",
                           name="vext_sa")

        def vproj_sa(tcx, oh):
            ps = mm_ps.tile([128, 512], F32, tag="mm", name="ps_v")
            for ic in range(WC):
                nc.tensor.matmul(
                    ps[:, 0:512],
                    qT[:, ic, tcx * 128:(tcx + 1) * 128],
                    wvt[:, oh * 4:(oh + 1) * 4, ic, :],
                    start=(ic == 0), stop=(ic == WC - 1))
            nh0 = oh * (512 // HD)
            nc.vector.tensor_copy(
                vext_all[:, tcx, nh0:nh0 + 512 // HD, 0:HD],
                ps[:, 0:512].rearrange("p (h d) -> p h d", d=HD))
            if oh == W // 512 - 1:
                nc.vector.tensor_copy(vext_all[:, tcx, :, HD],
                                      ones_f[:, 0:NH])

        osaT = sa.tile([128, WC, T], BF16, tag="osaT", name="osaT")
        x1pre = sa.tile([128, WC, T], BF16, tag="x1pre", name="x1pre")

        def oproj_sa(b, ob):
            ps = mm_ps.tile([128, NQ], F32, tag="mm", name="ps_o")
            gemm(ps[:, 0:NQ], wot, ob,
                 lambda ic: osaT[:, ic, b * NQ:(b + 1) * NQ], WC)
            nc.vector.tensor_add(x1pre[:, ob, b * NQ:(b + 1) * NQ],
                                 ps[:, 0:NQ],
                                 qT[:, ob, b * NQ:(b + 1) * NQ])

        # Interleaved SA attention: V-projection groups hide exp(b0);
        # scores(b1) hide exp while PV(b0) drains; out-proj(b0) groups hide
        # exp while PV(b1) drains. (Same HAM-warmth trick as the CA loop.)
        es0 = {}
        for hp in range(NH // 2):
            vproj_sa(hp % TC, hp // TC)
            es0[hp] = sa_scores_pair(0, ksaT, qsaT, hp)
        es1 = {}
        for hp in range(NH // 2):
            sa_pv_pair(0, vext_all, hp, es0[hp], osaT)
            es1[hp] = sa_scores_pair(1, ksaT, qsaT, hp)
        for hp in range(NH // 2):
            sa_pv_pair(1, vext_all, hp, es1[hp], osaT)
            oproj_sa(0, hp)
        for ob in range(WC):
            oproj_sa(1, ob)
        for b in range(B_loc):
            layernorm(lambda ic: x1pre[:, ic, b * NQ:(b + 1) * NQ], WC, NQ,
                      cols['ln1_g'], cols['ln1_b'],
                      lambda ic: x1T[:, ic, b * NQ:(b + 1) * NQ])

    # ================= cross-attention =================
    q2T = persist.tile([128, WC, T], BF16, tag="q2T", name="q2T")
    with nc.named_scope("ca"), \
         tc.tile_pool(name="ca_w", bufs=1) as ca_w, \
         tc.tile_pool(name="ca", bufs=1) as ca:
        with tc.tile_pool(name="ca_early", bufs=1) as cae:
            wqt2 = load_wT(cae, "wtQ", dram['ca_wqt'], "ca_wq_t")
            wvt2 = load_wT(ca_w, "wtV", dram['ca_wvt'], "ca_wv_t")
            wkt2 = load_wT(ca_w, "wtK", dram['ca_wkt'], "ca_wk_t")
            wot2 = load_wT(ca_w, "wtO", dram['ca_wot'], "ca_wo_t")
            x1pT = cae.tile([128, WC, T], BF16, tag="x1pT", name="x1pT")
            nc.vector.tensor_add(x1pT[:], x1T[:], peT[:])
            for ob in range(WC):
                ps = mm_ps.tile([128, T], F32, tag="mm", name="ps_q2")
                gemm(ps[:, 0:T], wqt2, ob, lambda ic: x1pT[:, ic, :], WC)
                nc.vector.tensor_copy(q2T[:, ob, :], ps[:, 0:T])

        ocaT = ca.tile([128, WC, T], BF16, tag="ocaT", name="ocaT")
        x2pre = ca.tile([128, WC, T], BF16, tag="x2pre", name="x2pre")

        def oproj_ca(b):
            """CA out-proj + residual for one batch (issued right after its
            normalize, so it lands in the pipelined region, not the tail)."""
            for ob in range(WC):
                ps = mm_ps.tile([128, NQ], F32, tag="mm", name="ps_o2")
                gemm(ps[:, 0:NQ], wot2, ob,
                     lambda ic: ocaT[:, ic, b * NQ:(b + 1) * NQ], WC)
                nc.vector.tensor_add(x2pre[:, ob, b * NQ:(b + 1) * NQ],
                                     ps[:, 0:NQ],
                                     x1T[:, ob, b * NQ:(b + 1) * NQ])

        with tc.tile_pool(name="ca_acc", bufs=1) as cacc, \
             tc.tile_pool(name="ca_jc", bufs=1) as cjc:
            # Software-pipelined chunk loop (see module docstring).
            acc = cacc.tile([HD + 1, NH, NQ], F32, tag="acc", name="acc_ca")
            n_chunks = B_loc * NJC

            def load_chunk(c):
                b, jc = c // NJC, c % NJC
                mT = cjc.tile([128, WC, JC], BF16, tag="mT", bufs=2,
                              name="mT")
                nc.sync.dma_start(
                    mT[:], dram['mT'][:, :, b * S + jc * JC:
                                      b * S + (jc + 1) * JC])
                k2T = cjc.tile([128, WC, JC], BF16, tag="k2T", bufs=2,
                               name="k2T")
                vext = cjc.tile([128, JSC, NH, HD + 1], BF16, tag="vext",
                                bufs=2, name="vext_ca")
                return mT, k2T, vext

            def proj_closures(mT, k2T, vext):
                cls = []
                for ob in range(WC):
                    def kproj(ob=ob, mT=mT, k2T=k2T):
                        ps = mm_ps.tile([128, JC], F32, tag="mm",
                                        name="ps_k2")
                        gemm(ps[:, 0:JC], wkt2, ob,
                             lambda ic: mT[:, ic, :], WC)
                        nc.vector.tensor_copy(k2T[:, ob, :], ps[:, 0:JC])
                    cls.append(kproj)
                for sj in range(JSC):
                    for oh in range(W // 512):
                        def vproj(sj=sj, oh=oh, mT=mT, vext=vext):
                            ps = mm_ps.tile([128, 512], F32, tag="mm",
                                            name="ps_v2")
                            for ic in range(WC):
                                nc.tensor.matmul(
                                    ps[:, 0:512],
                                    mT[:, ic, sj * 128:(sj + 1) * 128],
                                    wvt2[:, oh * 4:(oh + 1) * 4, ic, :],
                                    start=(ic == 0), stop=(ic == WC - 1))
                            nh0 = oh * (512 // HD)
                            nc.vector.tensor_copy(
                                vext[:, sj, nh0:nh0 + 512 // HD, 0:HD],
                                ps[:, 0:512].rearrange("p (h d) -> p h d",
                                                       d=HD))
                            if oh == W // 512 - 1:
                                nc.vector.tensor_copy(vext[:, sj, :, HD],
                                                      ones_f[:, 0:NH])
                        cls.append(vproj)
                return cls

            def sc_closure(b, k2T, hp):
                es = []
                for sub in range(2):
                    h = 2 * hp + sub
                    off = (h % HPC) * HD
                    fc = h // HPC
                    e = scratch.tile([128, JSC, NQ], BF16, tag="exp",
                                     bufs=4, name="e",
                                     padded_shape=[128, 4, NQ])
                    for half in range(JSC // 2):
                        js0 = half * 2
                        ps_s = sc_ps.tile([128, 2, NQ], F32, tag="sc",
                                          name="ps_sc")
                        for s2 in range(2):
                            js = js0 + s2
                            nc.tensor.matmul(
                                ps_s[:, s2, :],
                                k2T[off:off + HD, fc,
                                    js * 128:(js + 1) * 128],
                                q2T[off:off + HD, fc, b * NQ:(b + 1) * NQ])
                        nc.scalar.activation(e[:, js0:js0 + 2, :],
                                             ps_s[:, :, :], AF.Exp,
                                             scale=SCALE)
                    es.append(e)
                return es

            def pv_closure(vext, hp, es, first):
                ps_o = pv_ps.tile([HD + 1, 2, NQ], F32, tag="pv",
                                  name="ps_pv2")
                for sub, e in ((0, es[0]), (1, es[1])):
                    h = 2 * hp + sub
                    for js in range(JSC):
                        nc.tensor.matmul(ps_o[0:HD + 1, sub, :],
                                         vext[:, js, h, :], e[:, js, :],
                                         start=(js == 0),
                                         stop=(js == JSC - 1))
                if first:
                    nc.vector.tensor_copy(
                        acc[0:HD + 1, 2 * hp:2 * hp + 2, :],
                        ps_o[0:HD + 1, :, :])
                else:
                    nc.vector.tensor_add(
                        acc[0:HD + 1, 2 * hp:2 * hp + 2, :],
                        acc[0:HD + 1, 2 * hp:2 * hp + 2, :],
                        ps_o[0:HD + 1, :, :])

            prev = None          # (b, k2T, vext, first, jc) of chunk c-1
            cur = load_chunk(0)
            for c in range(n_chunks + 1):
                nxt = load_chunk(c + 1) if c + 1 < n_chunks else None
                projs = []
                if c < n_chunks:
                    mT, k2T, vext = cur
                    projs = proj_closures(mT, k2T, vext)
                # interleave: 2 proj groups, then scores(hp), then PV(hp-1)
                pend = None      # (hp, es) awaiting PV
                pi = 0
                for hp in range(NH // 2 + 1):
                    if pi < len(projs):
                        projs[pi]()
                        pi += 1
                    if hp < NH // 2 and prev is not None:
                        es = sc_closure(prev[0], prev[1], hp)
                    else:
                        es = None
                    if pi < len(projs):
                        projs[pi]()
                        pi += 1
                    if pend is not None:
                        pv_closure(prev[2], pend[0], pend[1], prev[3])
                    pend = (hp, es) if es is not None else None
                while pi < len(projs):
                    projs[pi]()
                    pi += 1
                if prev is not None:
                    if prev[4] == NJC - 1:   # last chunk of its batch
                        b_done = prev[0]
                        for h in range(NH):
                            normalize_head(h, acc[:, h, :], ocaT,
                                           b_done * NQ)
                        oproj_ca(b_done)
                if c < n_chunks:
                    prev = (c // NJC, k2T, vext, (c % NJC) == 0, c % NJC)
                    cur = nxt

        x2T = persist.tile([128, WC, T], BF16, tag="x2T", name="x2T")
        layernorm(lambda ic: x2pre[:, ic, :], WC, T,
                  cols['ln2_g'], cols['ln2_b'],
                  lambda ic: x2T[:, ic, :])

    # ================= FFN =================
    with nc.named_scope("ffn"), tc.tile_pool(name="ffn", bufs=1) as ffn:
        hT = ffn.tile([128, MC, T], BF16, tag="hT", name="hT")
        for oc in range(MC):
            w1t = ffn.tile([128, WC, 128], BF16, tag="w1t", bufs=4,
                           name="w1t")
            nc.sync.dma_start(w1t[:], dram['ffn_w1t'][:, oc])
            ps = mm_ps.tile([128, T], F32, tag="mm", name="ps_h")
            for ic in range(WC):
                nc.tensor.matmul(ps[:, 0:T], w1t[:, ic, :], x2T[:, ic, :],
                                 start=(ic == 0), stop=(ic == WC - 1))
            nc.scalar.activation(hT[:, oc, :], ps[:, 0:T], AF.Relu,
                                 bias=b1_col[:, oc:oc + 1])
        x3pre = ffn.tile([128, WC, T], BF16, tag="x3pre", name="x3pre")
        for ob in range(WC):
            w2t = ffn.tile([128, MC, 128], BF16, tag="w2t", bufs=2,
                           name="w2t")
            nc.sync.dma_start(w2t[:], dram['ffn_w2t'][:, ob])
            ps = mm_ps.tile([128, T], F32, tag="mm", name="ps_f")
            for ic in range(MC):
                nc.tensor.matmul(ps[:, 0:T], w2t[:, ic, :], hT[:, ic, :],
                                 start=(ic == 0), stop=(ic == MC - 1))
            tmp = scratch.tile([128, T], F32, tag="ftmp", bufs=1,
                               name="f_tmp")
            nc.scalar.activation(tmp[:, 0:T], ps[:, 0:T], AF.Identity,
                                 bias=cols['b2'][:, ob:ob + 1])
            nc.vector.tensor_add(x3pre[:, ob, :], tmp[:, 0:T], x2T[:, ob, :])
        x3T = ffn.tile([128, WC, T], BF16, tag="x3T", name="x3T")
        layernorm(lambda ic: x3pre[:, ic, :], WC, T,
                  cols['ln3_g'], cols['ln3_b'],
                  lambda ic: x3T[:, ic, :])
        # bf16 output: XBAR-transpose back to token-major, host casts to f32
        for tcx in range(TC):
            o_tm = ffn.tile([128, W], BF16, tag="o_tm", bufs=2, name="o_tm")
            for g in range(WC):
                nc.sync.dma_start_transpose(
                    o_tm[:, g * 128:(g + 1) * 128].rearrange(
                        "p (a n) -> p a n", a=1),
                    x3T[:, g, tcx * 128:(tcx + 1) * 128])
            nc.sync.dma_start(out_flat[tcx * 128:(tcx + 1) * 128, :], o_tm[:])

    return out_d


_PROGRAM_CACHE = {}


def _get_program(B_loc, NQ, S, W, NH, MLP, JC=512, repeat=1):
    key = (B_loc, NQ, S, W, NH, MLP, JC, repeat)
    if key not in _PROGRAM_CACHE:
        nc = bacc.Bacc("TRN2", target_bir_lowering=False, debug=False)
        with tile.TileContext(nc) as tc, \
             nc.allow_low_precision(reason="bf16 matmul pipeline"):
            for r in range(repeat):
                with ExitStack() as ctx:
                    build_decoder(nc, tc, ctx, B_loc, NQ, S, W, NH, MLP, JC,
                                  suffix=("" if r == 0 else f"_r{r}"))
        nc.compile()
        _PROGRAM_CACHE[key] = nc
    return _PROGRAM_CACHE[key]


def _wt_blockmajor(w):
    """W [O, I] fp32 -> bf16 W^T block-major [128, O/128, I/128, 128]:
    wt[p, ob, c, n] = W[ob*128+n, c*128+p]."""
    O, I = w.shape
    # [ob, n, c, p] -> transpose to [p, ob, c, n]
    v = w.reshape(O // 128, 128, I // 128, 128).transpose(3, 0, 2, 1)
    return np.ascontiguousarray(v.astype(BF16_NP))


def _featmajor(x):
    """x [B, N, W] fp32 -> bf16 feature-major [128, W/128, B*N]:
    v[p, c, t] = x[b(t), n(t), c*128+p]."""
    B, N, W = x.shape
    v = x.reshape(B * N, W // 128, 128).transpose(2, 1, 0)
    return np.ascontiguousarray(v.astype(BF16_NP))


def _col128(v):
    """[n] fp32 -> [128, n/128] per-partition columns: c[p, i] = v[i*128+p]."""
    return np.ascontiguousarray(v.reshape(-1, 128).T.astype(np.float32))


def _make_in_maps(inputs):
    B, NQ, W = inputs['query'].shape
    S = inputs['enc_mem'].shape[1]
    MLP = inputs['ffn_w1'].shape[0]
    B_loc = B // N_CORES
    f32 = {k: np.asarray(v, dtype=np.float32) for k, v in inputs.items()}

    shared = {}
    for p in ('sa', 'ca'):
        for k in 'qkvo':
            shared[f'{p}_w{k}t'] = _wt_blockmajor(f32[f'{p}_w{k}'])
    # ffn_w1t [128, MC, WC, 128]: w1t[p, oc, c, n] = W1[oc*128+n, c*128+p]
    w1 = _wt_blockmajor(f32['ffn_w1'])            # [128, MC, WC, 128]
    shared['ffn_w1t'] = w1
    # ffn_w2t [128, WC, MC, 128]
    shared['ffn_w2t'] = _wt_blockmajor(f32['ffn_w2'])
    for i in (1, 2, 3):
        for gb in ('g', 'b'):
            shared[f'ln{i}_{gb}_col'] = _col128(f32[f'ln{i}_{gb}'])
    shared['b1_col'] = _col128(f32['ffn_b1'])
    shared['b2_col'] = _col128(f32['ffn_b2'])

    q, pe, m = f32['query'], f32['out_pos_enc'], f32['enc_mem']
    qk = q + pe
    in_maps = []
    for c in range(N_CORES):
        sl = slice(c * B_loc, (c + 1) * B_loc)
        mp = dict(shared)
        mp['qT'] = _featmajor(q[sl])
        mp['qkT'] = _featmajor(qk[sl])
        mp['peT'] = _featmajor(pe[sl])
        mp['mT'] = _featmajor(m[sl])
        in_maps.append(mp)
    return in_maps


def kernel(**inputs):
    B, NQ, W = inputs['query'].shape
    S = inputs['enc_mem'].shape[1]
    MLP = inputs['ffn_w1'].shape[0]
    NH = 16
    assert B % N_CORES == 0
    B_loc = B // N_CORES

    nc = _get_program(B_loc, NQ, S, W, NH, MLP)
    in_maps = _make_in_maps(inputs)

    res = run_bass_kernel_spmd(nc, in_maps, list(range(N_CORES)))
    return np.concatenate(
        [np.asarray(res.results[c]["out"]).astype(np.float32)
         for c in range(N_CORES)], axis=0)


# revision 64
# speedup vs baseline: 1.0178x; 1.0178x over previous
"""Trainium2 Bass kernel for nn_DecoderLayer (dense transformer decoder layer).

Sharding: data-parallel over batch (16 batches -> 8 cores x 2 each). Each core
runs the full decoder layer on its batch slice; no collectives.

v7: all weight/input layout transformation happens host-side in kernel()
(numpy): weights are pre-transposed to W^T block-major bf16, enc_mem / query /
pos-enc are pre-transposed to feature-major bf16, and q+pos_enc is
pre-summed. The device program is then pure GEMM streaming: the SP queue
carries only plain contiguous DMAs, ACT only exp/LN/relu, DVE only PSUM
evictions and element-wise adds. This avoids the on-chip DMA->cast->XBAR
producer chains whose in-order queue waits kept head-of-line-blocking the
enc-chunk loads and starving the PE (HAM then re-throttles it to 1.2 GHz,
doubling every matmul).

Layout: activations are feature-major ("xT": [feature partitions, token
free]) so every linear is psum = W^T.T @ xT with bf16 operands and fp32 PSUM
accumulation. Attention uses transposed scores S^T = K_hT.T @ Q_hT
([j partitions, i free]), exp without max-subtraction (scores are bounded),
and a ones-column appended to V so the softmax denominator comes out of the
same PV matmul. LayerNorm runs feature-major with partition sums via
ones-vector matmuls and per-token broadcast via K=1 matmuls.

The cross-attention main loop is software-pipelined: chunk c's K/V-projection
matmul groups interleave with chunk c-1's attention at head-pair granularity
(scores and PV split one slot apart), so the PE always has projection work
while ACT grinds the exps and HAM stays at full clock.
"""
import sys
import numpy as np

sys.path.insert(0, '/opt/trn_rl_repo')

import ml_dtypes  # noqa: E402
import concourse.bass as bass  # noqa: E402
import concourse.tile as tile  # noqa: E402
from concourse import bacc, mybir  # noqa: E402
from concourse.bass_utils import run_bass_kernel_spmd  # noqa: E402
from concourse.masks import make_identity  # noqa: E402
from contextlib import ExitStack  # noqa: E402

F32 = mybir.dt.float32
BF16 = mybir.dt.bfloat16
AF = mybir.ActivationFunctionType
BF16_NP = ml_dtypes.bfloat16

EPS = 1e-5
N_CORES = 8


def build_decoder(nc, tc, ctx, B_loc, NQ, S, W, NH, MLP, JC=512, suffix=""):
    HD = W // NH
    assert HD == 64 and NQ % 128 == 0 and W % 512 == 0 and JC % 128 == 0
    T = B_loc * NQ          # decoder tokens per core
    TC = T // 128
    WC = W // 128
    MC = MLP // 128
    NJC = S // JC           # enc chunks per batch
    JSC = JC // 128
    NQC = NQ // 128
    SCALE = float(W) ** -0.5
    HPC = 128 // HD         # heads per feature chunk (2)

    # Transformed inputs (host-side marshalling):
    #   *_wt:  [128, O/128, I/128, 128] bf16, wt[p,ob,c,n] = W[ob*128+n, c*128+p]
    #   qT/qkT/peT: [128, WC, T] bf16 feature-major (qkT = q + pos_enc)
    #   mT:    [128, WC, T_enc] bf16 feature-major (T_enc = B_loc*S)
    #   *_col: [128, n/128] f32 per-partition param columns
    dram = {}
    for name, shape, dt in (
        [('qT', [128, WC, T], BF16), ('qkT', [128, WC, T], BF16),
         ('peT', [128, WC, T], BF16), ('mT', [128, WC, B_loc * S], BF16),
         ('ffn_w1t', [128, MC, WC, 128], BF16),
         ('ffn_w2t', [128, WC, MC, 128], BF16),
         ('b1_col', [128, MC], F32), ('b2_col', [128, WC], F32)] +
        [(f'{p}_w{k}t', [128, WC, WC, 128], BF16)
         for p in ('sa', 'ca') for k in 'qkvo'] +
        [(f'ln{i}_{gb}_col', [128, WC], F32)
         for i in (1, 2, 3) for gb in ('g', 'b')]
    ):
        if suffix:
            dram[name] = build_decoder._dram_cache[name]
        else:
            dram[name] = nc.dram_tensor(name, shape, dt, kind="ExternalInput")
    build_decoder._dram_cache = dict(dram)
    out_d = nc.dram_tensor("out" + suffix, [B_loc, NQ, W], F32,
                           kind="ExternalOutput")
    out_flat = out_d.rearrange("b n w -> (b n) w")

    # ---------------- global pools ----------------
    consts = ctx.enter_context(tc.tile_pool(name="consts", bufs=1))
    persist = ctx.enter_context(tc.tile_pool(name="persist", bufs=1))
    scratch = ctx.enter_context(tc.tile_pool(name="scratch", bufs=2))
    mm_ps = ctx.enter_context(tc.tile_pool(name="mm_ps", bufs=3, space="PSUM"))
    sc_ps = ctx.enter_context(tc.tile_pool(name="sc_ps", bufs=2, space="PSUM"))
    pv_ps = ctx.enter_context(tc.tile_pool(name="pv_ps", bufs=3, space="PSUM"))

    ident = consts.tile([128, 128], F32, tag="ident")
    make_identity(nc, ident[:])
    ones_f = consts.tile([128, 128], F32, tag="ones_f")
    nc.gpsimd.memset(ones_f[:], 1.0)
    ones_b = consts.tile([128, 128], BF16, tag="ones_b")
    nc.vector.tensor_copy(ones_b[:], ones_f[:])
    eps_t = consts.tile([1, 1], F32, tag="eps")
    nc.gpsimd.memset(eps_t[:], EPS)

    cols = {}
    for name in ['ln1_g', 'ln1_b', 'ln2_g', 'ln2_b', 'ln3_g', 'ln3_b',
                 'b2']:
        cn = name + '_col' if name != 'b2' else 'b2_col'
        t = consts.tile([128, WC], F32, tag=cn, name=cn)
        nc.sync.dma_start(t[:], dram[cn][:, :])
        cols[name] = t
    b1_col = consts.tile([128, MC], F32, tag="b1c", name="b1_col")
    nc.sync.dma_start(b1_col[:], dram['b1_col'][:, :])

    # ---------------- helpers ----------------
    def load_wT(pool, tag, wt_d, name, bufs=1):
        """bf16 W^T tile from pre-transposed DRAM; per-ob DMAs so the first
        GEMM group can start after one 256KB transfer."""
        OB = wt_d.shape[1]
        wt = pool.tile([128, OB, wt_d.shape[2], 128], BF16, tag=tag,
                       name=name, bufs=bufs)
        for ob in range(OB):
            nc.sync.dma_start(wt[:, ob], wt_d[:, ob])
        return wt

    def gemm(psum, wt, ob, rhs, ICn):
        """psum[oc 128, N] += sum_ic wt[:, ob, ic, :].T @ rhs(ic)."""
        for ic in range(ICn):
            nc.tensor.matmul(psum, wt[:, ob, ic, :], rhs(ic),
                             start=(ic == 0), stop=(ic == ICn - 1))

    def layernorm(x_fn, n_chunks, N, g_col, b_col, out_fn):
        """Feature-major LN over the partition (feature) dim."""
        ps_s = sc_ps.tile([1, N], F32, tag="sc", name="ps_s",
                          padded_shape=[1, 512])
        for ic in range(n_chunks):
            nc.tensor.matmul(ps_s[0:1, :], ones_b[:, 0:1], x_fn(ic),
                             start=(ic == 0), stop=(ic == n_chunks - 1))
        ps_q = sc_ps.tile([1, N], F32, tag="sc", name="ps_q",
                          padded_shape=[1, 512])
        for ic in range(n_chunks):
            sq = scratch.tile([128, N], BF16, tag="sq", name="sq",
                              padded_shape=[128, 512])
            nc.vector.tensor_mul(sq[:, 0:N], x_fn(ic), x_fn(ic))
            nc.tensor.matmul(ps_q[0:1, :], ones_b[:, 0:1], sq[:, 0:N],
                             start=(ic == 0), stop=(ic == n_chunks - 1))
        inv_w = 1.0 / (n_chunks * 128)
        mu = scratch.tile([1, N], BF16, tag="st_mu", bufs=1, name="mu")
        nc.scalar.activation(mu[0:1, :], ps_s[0:1, :], AF.Copy, scale=inv_w)
        ex2 = scratch.tile([1, N], F32, tag="st_e", bufs=1, name="ex2")
        nc.scalar.activation(ex2[0:1, :], ps_q[0:1, :], AF.Copy, scale=inv_w)
        mu2 = scratch.tile([1, N], F32, tag="st_x", bufs=1, name="mu2")
        nc.vector.tensor_mul(mu2[0:1, :], mu[0:1, :], mu[0:1, :])
        var = scratch.tile([1, N], F32, tag="st_v", bufs=1, name="var")
        nc.vector.tensor_sub(var[0:1, :], ex2[0:1, :], mu2[0:1, :])
        sd = scratch.tile([1, N], F32, tag="st_x", bufs=1, name="sd")
        nc.scalar.activation(sd[0:1, :], var[0:1, :], AF.Sqrt,
                             bias=eps_t[0:1, 0:1])
        rstd = scratch.tile([1, N], BF16, tag="st_r", bufs=1, name="rstd")
        nc.vector.reciprocal(rstd[0:1, :], sd[0:1, :])
        ps_mu = sc_ps.tile([128, N], F32, tag="sc", name="ps_mu",
                           padded_shape=[128, 512])
        nc.tensor.matmul(ps_mu[:, 0:N], ones_b[0:1, :], mu[0:1, :])
        ps_rs = sc_ps.tile([128, N], F32, tag="sc", name="ps_rs",
                           padded_shape=[128, 512])
        nc.tensor.matmul(ps_rs[:, 0:N], ones_b[0:1, :], rstd[0:1, :])
        for ic in range(n_chunks):
            xm = scratch.tile([128, N], F32, tag="xm", bufs=1, name="xm",
                              padded_shape=[128, 512])
            nc.vector.tensor_sub(xm[:, 0:N], x_fn(ic), ps_mu[:, 0:N])
            nc.vector.tensor_mul(xm[:, 0:N], xm[:, 0:N], ps_rs[:, 0:N])
            nc.scalar.activation(out_fn(ic), xm[:, 0:N], AF.Identity,
                                 bias=b_col[:, ic:ic + 1],
                                 scale=g_col[:, ic:ic + 1])

    def normalize_head(h, src, oT, col):
        """oT head slice = src[0:HD] / src[HD] (softmax sums row)."""
        off = (h % HPC) * HD
        fc = h // HPC
        rec = scratch.tile([1, NQ], BF16, tag="rec", bufs=2, name="rec")
        nc.vector.reciprocal(rec[0:1, :], src[HD:HD + 1, :])
        ps_b = mm_ps.tile([HD, NQ], F32, tag="mm", name="ps_bc")
        nc.tensor.matmul(ps_b[0:HD, :], ones_b[0:1, 0:HD], rec[0:1, :])
        nc.vector.tensor_mul(oT[off:off + HD, fc, col:col + NQ], src[0:HD, :],
                             ps_b[0:HD, :])

    def sa_scores_pair(b, ksaT, qsaT, hp):
        """Scores + exp for head pair hp of batch b (SA, NQC key blocks)."""
        es = []
        for sub in range(2):
            h = 2 * hp + sub
            off = (h % HPC) * HD
            fc = h // HPC
            e = sa_pool[0].tile([128, NQC, NQ], BF16, tag="expsa",
                                bufs=16, name="esa")
            ps_s = sc_ps.tile([128, 2, NQ], F32, tag="sc", name="ps_sc")
            for js in range(NQC):
                nc.tensor.matmul(
                    ps_s[:, js, :],
                    ksaT[off:off + HD, fc, b * NQ + js * 128:
                         b * NQ + (js + 1) * 128],
                    qsaT[off:off + HD, fc, b * NQ:(b + 1) * NQ])
            nc.scalar.activation(e[:, :, :], ps_s[:, 0:NQC, :], AF.Exp,
                                 scale=SCALE)
            es.append(e)
        return es

    def sa_pv_pair(b, vext_all, hp, es, oT):
        ps_o = pv_ps.tile([HD + 1, 2, NQ], F32, tag="pv", name="ps_pv2")
        for sub, e in ((0, es[0]), (1, es[1])):
            h = 2 * hp + sub
            for js in range(NQC):
                nc.tensor.matmul(
                    ps_o[0:HD + 1, sub, :],
                    vext_all[:, b * NQC + js, h, :], e[:, js, :],
                    start=(js == 0), stop=(js == NQC - 1))
        pv_sb = sa_pool[0].tile([HD + 1, 2, NQ], F32, tag="pvsb", bufs=2,
                                name="pv_sb")
        nc.vector.tensor_copy(pv_sb[:], ps_o[:])
        normalize_head(2 * hp, pv_sb[:, 0, :], oT, b * NQ)
        normalize_head(2 * hp + 1, pv_sb[:, 1, :], oT, b * NQ)

    # ================= P0: self-attention =================
    sa_pool = [None]
    x1T = persist.tile([128, WC, T], BF16, tag="x1T", name="x1T")
    qT = persist.tile([128, WC, T], BF16, tag="qT", name="qT")
    peT = persist.tile([128, WC, T], BF16, tag="peT", name="peT")
    with nc.named_scope("sa"), \
         tc.tile_pool(name="sa_w", bufs=1) as sa_w, \
         tc.tile_pool(name="sa", bufs=1) as sa:
        sa_pool[0] = sa
        qkT = sa.tile([128, WC, T], BF16, tag="qkT", name="qkT")
        # startup order: first Q-GEMM needs wqt[ob0] + qkT, so those DMAs
        # go first; qT/peT (V-proj / residual / CA) trail behind.
        wqt = load_wT(sa_w, "wtA", dram['sa_wqt'], "sa_wq_t", bufs=2)
        nc.sync.dma_start(qkT[:], dram['qkT'][:, :, :])
        nc.sync.dma_start(qT[:], dram['qT'][:, :, :])
        nc.sync.dma_start(peT[:], dram['peT'][:, :, :])
        qsaT = sa.tile([128, WC, T], BF16, tag="big", bufs=3, name="qsaT")
        for ob in range(WC):
            ps = mm_ps.tile([128, T], F32, tag="mm", name="ps_q")
            gemm(ps[:, 0:T], wqt, ob, lambda ic: qkT[:, ic, :], WC)
            nc.vector.tensor_copy(qsaT[:, ob, :], ps[:, 0:T])
        wkt = load_wT(sa_w, "wtA", dram['sa_wkt'], "sa_wk_t", bufs=2)
        ksaT = sa.tile([128, WC, T], BF16, tag="big", bufs=3, name="ksaT")
        for ob in range(WC):
            ps = mm_ps.tile([128, T], F32, tag="mm", name="ps_k")
            gemm(ps[:, 0:T], wkt, ob, lambda ic: qkT[:, ic, :], WC)
            nc.vector.tensor_copy(ksaT[:, ob, :], ps[:, 0:T])
        wvt = load_wT(sa_w, "wtA", dram['sa_wvt'], "sa_wv_t", bufs=2)
        wot = load_wT(sa_w, "wtA", dram['sa_wot'], "sa_wo_t", bufs=2)
        vext_all = sa.tile([128, TC, NH, HD + 1], BF16, tag="vext",
                           name="vext_sa")

        def vproj_sa(tcx, oh):
            ps = mm_ps.tile([128, 512], F32, tag="mm", name="ps_v")
            for ic in range(WC):
                nc.tensor.matmul(
                    ps[:, 0:512],
                    qT[:, ic, tcx * 128:(tcx + 1) * 128],
                    wvt[:, oh * 4:(oh + 1) * 4, ic, :],
                    start=(ic == 0), stop=(ic == WC - 1))
            nh0 = oh * (512 // HD)
            nc.vector.tensor_copy(
                vext_all[:, tcx, nh0:nh0 + 512 // HD, 0:HD],
                ps[:, 0:512].rearrange("p (h d) -> p h d", d=HD))
            if oh == W // 512 - 1:
                nc.vector.tensor_copy(vext_all[:, tcx, :, HD],
                                      ones_f[:, 0:NH])

        osaT = sa.tile([128, WC, T], BF16, tag="osaT", name="osaT")
        x1pre = sa.tile([128, WC, T], BF16, tag="x1pre", name="x1pre")

        def oproj_sa(b, ob):
            ps = mm_ps.tile([128, NQ], F32, tag="mm", name="ps_o")
            gemm(ps[:, 0:NQ], wot, ob,
                 lambda ic: osaT[:, ic, b * NQ:(b + 1) * NQ], WC)
            nc.vector.tensor_add(x1pre[:, ob, b * NQ:(b + 1) * NQ],
                                 ps[:, 0:NQ],
                                 qT[:, ob, b * NQ:(b + 1) * NQ])

        # Interleaved SA attention: V-projection groups hide exp(b0);
        # scores(b1) hide exp while PV(b0) drains; out-proj(b0) groups hide
        # exp while PV(b1) drains. (Same HAM-warmth trick as the CA loop.)
        es0 = {}
        for hp in range(NH // 2):
            vproj_sa(hp % TC, hp // TC)
            es0[hp] = sa_scores_pair(0, ksaT, qsaT, hp)
        es1 = {}
        for hp in range(NH // 2):
            sa_pv_pair(0, vext_all, hp, es0[hp], osaT)
            es1[hp] = sa_scores_pair(1, ksaT, qsaT, hp)
        for hp in range(NH // 2):
            sa_pv_pair(1, vext_all, hp, es1[hp], osaT)
            oproj_sa(0, hp)
        for ob in range(WC):
            oproj_sa(1, ob)
        for b in range(B_loc):
            layernorm(lambda ic: x1pre[:, ic, b * NQ:(b + 1) * NQ], WC, NQ,
                      cols['ln1_g'], cols['ln1_b'],
                      lambda ic: x1T[:, ic, b * NQ:(b + 1) * NQ])

    # ================= cross-attention =================
    q2T = persist.tile([128, WC, T], BF16, tag="q2T", name="q2T")
    with nc.named_scope("ca"), \
         tc.tile_pool(name="ca_w", bufs=1) as ca_w, \
         tc.tile_pool(name="ca", bufs=1) as ca:
        with tc.tile_pool(name="ca_early", bufs=1) as cae:
            wqt2 = load_wT(cae, "wtQ", dram['ca_wqt'], "ca_wq_t")
            wvt2 = load_wT(ca_w, "wtV", dram['ca_wvt'], "ca_wv_t")
            wkt2 = load_wT(ca_w, "wtK", dram['ca_wkt'], "ca_wk_t")
            wot2 = load_wT(ca_w, "wtO", dram['ca_wot'], "ca_wo_t")
            x1pT = cae.tile([128, WC, T], BF16, tag="x1pT", name="x1pT")
            nc.vector.tensor_add(x1pT[:], x1T[:], peT[:])
            for ob in range(WC):
                ps = mm_ps.tile([128, T], F32, tag="mm", name="ps_q2")
                gemm(ps[:, 0:T], wqt2, ob, lambda ic: x1pT[:, ic, :], WC)
                nc.vector.tensor_copy(q2T[:, ob, :], ps[:, 0:T])

        ocaT = ca.tile([128, WC, T], BF16, tag="ocaT", name="ocaT")
        x2pre = ca.tile([128, WC, T], BF16, tag="x2pre", name="x2pre")

        def oproj_ca(b):
            """CA out-proj + residual for one batch (issued right after its
            normalize, so it lands in the pipelined region, not the tail)."""
            for ob in range(WC):
                ps = mm_ps.tile([128, NQ], F32, tag="mm", name="ps_o2")
                gemm(ps[:, 0:NQ], wot2, ob,
                     lambda ic: ocaT[:, ic, b * NQ:(b + 1) * NQ], WC)
                nc.vector.tensor_add(x2pre[:, ob, b * NQ:(b + 1) * NQ],
                                     ps[:, 0:NQ],
                                     x1T[:, ob, b * NQ:(b + 1) * NQ])

        with tc.tile_pool(name="ca_acc", bufs=1) as cacc, \
             tc.tile_pool(name="ca_jc", bufs=1) as cjc:
            # Software-pipelined chunk loop (see module docstring).
            acc = cacc.tile([HD + 1, NH, NQ], F32, tag="acc", name="acc_ca")
            n_chunks = B_loc * NJC

            def load_chunk(c):
                b, jc = c // NJC, c % NJC
                mT = cjc.tile([128, WC, JC], BF16, tag="mT", bufs=2,
                              name="mT")
                nc.sync.dma_start(
                    mT[:], dram['mT'][:, :, b * S + jc * JC:
                                      b * S + (jc + 1) * JC])
                k2T = cjc.tile([128, WC, JC], BF16, tag="k2T", bufs=2,
                               name="k2T")
                vext = cjc.tile([128, JSC, NH, HD + 1], BF16, tag="vext",
                                bufs=2, name="vext_ca")
                return mT, k2T, vext

            def proj_closures(mT, k2T, vext):
                cls = []
                for ob in range(WC):
                    def kproj(ob=ob, mT=mT, k2T=k2T):
                        ps = mm_ps.tile([128, JC], F32, tag="mm",
                                        name="ps_k2")
                        gemm(ps[:, 0:JC], wkt2, ob,
                             lambda ic: mT[:, ic, :], WC)
                        nc.vector.tensor_copy(k2T[:, ob, :], ps[:, 0:JC])
                    cls.append(kproj)
                for sj in range(JSC):
                    for oh in range(W // 512):
                        def vproj(sj=sj, oh=oh, mT=mT, vext=vext):
                            ps = mm_ps.tile([128, 512], F32, tag="mm",
                                            name="ps_v2")
                            for ic in range(WC):
                                nc.tensor.matmul(
                                    ps[:, 0:512],
                                    mT[:, ic, sj * 128:(sj + 1) * 128],
                                    wvt2[:, oh * 4:(oh + 1) * 4, ic, :],
                                    start=(ic == 0), stop=(ic == WC - 1))
                            nh0 = oh * (512 // HD)
                            nc.vector.tensor_copy(
                                vext[:, sj, nh0:nh0 + 512 // HD, 0:HD],
                                ps[:, 0:512].rearrange("p (h d) -> p h d",
                                                       d=HD))
                            if oh == W // 512 - 1:
                                nc.vector.tensor_copy(vext[:, sj, :, HD],
                                                      ones_f[:, 0:NH])
                        cls.append(vproj)
                return cls

            def sc_closure(b, k2T, hp):
                es = []
                for sub in range(2):
                    h = 2 * hp + sub
                    off = (h % HPC) * HD
                    fc = h // HPC
                    e = scratch.tile([128, JSC, NQ], BF16, tag="exp",
                                     bufs=4, name="e",
                                     padded_shape=[128, 4, NQ])
                    for half in range(JSC // 2):
                        js0 = half * 2
                        ps_s = sc_ps.tile([128, 2, NQ], F32, tag="sc",
                                          name="ps_sc")
                        for s2 in range(2):
                            js = js0 + s2
                            nc.tensor.matmul(
                                ps_s[:, s2, :],
                                k2T[off:off + HD, fc,
                                    js * 128:(js + 1) * 128],
                                q2T[off:off + HD, fc, b * NQ:(b + 1) * NQ])
                        nc.scalar.activation(e[:, js0:js0 + 2, :],
                                             ps_s[:, :, :], AF.Exp,
                                             scale=SCALE)
                    es.append(e)
                return es

            def pv_closure(vext, hp, es, first):
                ps_o = pv_ps.tile([HD + 1, 2, NQ], F32, tag="pv",
                                  name="ps_pv2")
                for sub, e in ((0, es[0]), (1, es[1])):
                    h = 2 * hp + sub
                    for js in range(JSC):
                        nc.tensor.matmul(ps_o[0:HD + 1, sub, :],
                                         vext[:, js, h, :], e[:, js, :],
                                         start=(js == 0),
                                         stop=(js == JSC - 1))
                if first:
                    nc.vector.tensor_copy(
                        acc[0:HD + 1, 2 * hp:2 * hp + 2, :],
                        ps_o[0:HD + 1, :, :])
                else:
                    nc.vector.tensor_add(
                        acc[0:HD + 1, 2 * hp:2 * hp + 2, :],
                        acc[0:HD + 1, 2 * hp:2 * hp + 2, :],
                        ps_o[0:HD + 1, :, :])

            prev = None          # (b, k2T, vext, first, jc) of chunk c-1
            cur = load_chunk(0)
            for c in range(n_chunks + 1):
                nxt = load_chunk(c + 1) if c + 1 < n_chunks else None
                projs = []
                if c < n_chunks:
                    mT, k2T, vext = cur
                    projs = proj_closures(mT, k2T, vext)
                # interleave: 2 proj groups, then scores(hp), then PV(hp-1)
                pend = None      # (hp, es) awaiting PV
                pi = 0
                for hp in range(NH // 2 + 1):
                    if pi < len(projs):
                        projs[pi]()
                        pi += 1
                    if hp < NH // 2 and prev is not None:
                        es = sc_closure(prev[0], prev[1], hp)
                    else:
                        es = None
                    if pi < len(projs):
                        projs[pi]()
                        pi += 1
                    if pend is not None:
                        pv_closure(prev[2], pend[0], pend[1], prev[3])
                    pend = (hp, es) if es is not None else None
                while pi < len(projs):
                    projs[pi]()
                    pi += 1
                if prev is not None:
                    if prev[4] == NJC - 1:   # last chunk of its batch
                        b_done = prev[0]
                        for h in range(NH):
                            normalize_head(h, acc[:, h, :], ocaT,
                                           b_done * NQ)
                        oproj_ca(b_done)
                if c < n_chunks:
                    prev = (c // NJC, k2T, vext, (c % NJC) == 0, c % NJC)
                    cur = nxt

        x2T = persist.tile([128, WC, T], BF16, tag="x2T", name="x2T")
        layernorm(lambda ic: x2pre[:, ic, :], WC, T,
                  cols['ln2_g'], cols['ln2_b'],
                  lambda ic: x2T[:, ic, :])

    # ================= FFN =================
    with nc.named_scope("ffn"), tc.tile_pool(name="ffn", bufs=1) as ffn:
        hT = ffn.tile([128, MC, T], BF16, tag="hT", name="hT")
        for oc in range(MC):
            w1t = ffn.tile([128, WC, 128], BF16, tag="w1t", bufs=4,
                           name="w1t")
            nc.sync.dma_start(w1t[:], dram['ffn_w1t'][:, oc])
            ps = mm_ps.tile([128, T], F32, tag="mm", name="ps_h")
            for ic in range(WC):
                nc.tensor.matmul(ps[:, 0:T], w1t[:, ic, :], x2T[:, ic, :],
                                 start=(ic == 0), stop=(ic == WC - 1))
            nc.scalar.activation(hT[:, oc, :], ps[:, 0:T], AF.Relu,
                                 bias=b1_col[:, oc:oc + 1])
        x3pre = ffn.tile([128, WC, T], BF16, tag="x3pre", name="x3pre")
        for ob in range(WC):
            w2t = ffn.tile([128, MC, 128], BF16, tag="w2t", bufs=2,
                           name="w2t")
            nc.sync.dma_start(w2t[:], dram['ffn_w2t'][:, ob])
            ps = mm_ps.tile([128, T], F32, tag="mm", name="ps_f")
            for ic in range(MC):
                nc.tensor.matmul(ps[:, 0:T], w2t[:, ic, :], hT[:, ic, :],
                                 start=(ic == 0), stop=(ic == MC - 1))
            tmp = scratch.tile([128, T], F32, tag="ftmp", bufs=1,
                               name="f_tmp")
            nc.scalar.activation(tmp[:, 0:T], ps[:, 0:T], AF.Identity,
                                 bias=cols['b2'][:, ob:ob + 1])
            nc.vector.tensor_add(x3pre[:, ob, :], tmp[:, 0:T], x2T[:, ob, :])
        x3T = ffn.tile([128, WC, T], F32, tag="x3T", name="x3T")
        layernorm(lambda ic: x3pre[:, ic, :], WC, T,
                  cols['ln3_g'], cols['ln3_b'],
                  lambda ic: x3T[:, ic, :])
        # Output via PE transposes, g-major so each group pipelines right
        # behind its LN3 apply; per-tcx staging tiles let the 4 output DMAs
        # overlap the remaining transposes.
        o_tms = []
        for tcx in range(TC):
            o_tms.append(ffn.tile([128, W], F32, tag="o_tm", bufs=4,
                                  name="o_tm"))
        for g in range(WC // 4):
            for tcx in range(TC):
                pt = sc_ps.tile([128, 512], F32, tag="sc", name="pt_out")
                for k in range(4):
                    nc.tensor.transpose(
                        pt[:, k * 128:(k + 1) * 128],
                        x3T[:, g * 4 + k, tcx * 128:(tcx + 1) * 128],
                        ident[:])
                nc.vector.tensor_copy(
                    o_tms[tcx][:, g * 512:(g + 1) * 512], pt[:])
        for tcx in range(TC):
            nc.sync.dma_start(out_flat[tcx * 128:(tcx + 1) * 128, :],
                              o_tms[tcx][:])

    return out_d


_PROGRAM_CACHE = {}


def _get_program(B_loc, NQ, S, W, NH, MLP, JC=512, repeat=1):
    key = (B_loc, NQ, S, W, NH, MLP, JC, repeat)
    if key not in _PROGRAM_CACHE:
        nc = bacc.Bacc("TRN2", target_bir_lowering=False, debug=False)
        with tile.TileContext(nc) as tc, \
             nc.allow_low_precision(reason="bf16 matmul pipeline"):
            for r in range(repeat):
                with ExitStack() as ctx:
                    build_decoder(nc, tc, ctx, B_loc, NQ, S, W, NH, MLP, JC,
                                  suffix=("" if r == 0 else f"_r{r}"))
        nc.compile()
        _PROGRAM_CACHE[key] = nc
    return _PROGRAM_CACHE[key]


def _wt_blockmajor(w):
    """W [O, I] fp32 -> bf16 W^T block-major [128, O/128, I/128, 128]:
    wt[p, ob, c, n] = W[ob*128+n, c*128+p]."""
    O, I = w.shape
    # [ob, n, c, p] -> transpose to [p, ob, c, n]
    v = w.reshape(O // 128, 128, I // 128, 128).transpose(3, 0, 2, 1)
    return np.ascontiguousarray(v.astype(BF16_NP))


def _featmajor(x):
    """x [B, N, W] fp32 -> bf16 feature-major [128, W/128, B*N]:
    v[p, c, t] = x[b(t), n(t), c*128+p]."""
    B, N, W = x.shape
    v = x.reshape(B * N, W // 128, 128).transpose(2, 1, 0)
    return np.ascontiguousarray(v.astype(BF16_NP))


def _col128(v):
    """[n] fp32 -> [128, n/128] per-partition columns: c[p, i] = v[i*128+p]."""
    return np.ascontiguousarray(v.reshape(-1, 128).T.astype(np.float32))


def _make_in_maps(inputs):
    B, NQ, W = inputs['query'].shape
    S = inputs['enc_mem'].shape[1]
    MLP = inputs['ffn_w1'].shape[0]
    B_loc = B // N_CORES
    f32 = {k: np.asarray(v, dtype=np.float32) for k, v in inputs.items()}

    shared = {}
    for p in ('sa', 'ca'):
        for k in 'qkvo':
            shared[f'{p}_w{k}t'] = _wt_blockmajor(f32[f'{p}_w{k}'])
    # ffn_w1t [128, MC, WC, 128]: w1t[p, oc, c, n] = W1[oc*128+n, c*128+p]
    w1 = _wt_blockmajor(f32['ffn_w1'])            # [128, MC, WC, 128]
    shared['ffn_w1t'] = w1
    # ffn_w2t [128, WC, MC, 128]
    shared['ffn_w2t'] = _wt_blockmajor(f32['ffn_w2'])
    for i in (1, 2, 3):
        for gb in ('g', 'b'):
            shared[f'ln{i}_{gb}_col'] = _col128(f32[f'ln{i}_{gb}'])
    shared['b1_col'] = _col128(f32['ffn_b1'])
    shared['b2_col'] = _col128(f32['ffn_b2'])

    q, pe, m = f32['query'], f32['out_pos_enc'], f32['enc_mem']
    qk = q + pe
    in_maps = []
    for c in range(N_CORES):
        sl = slice(c * B_loc, (c + 1) * B_loc)
        mp = dict(shared)
        mp['qT'] = _featmajor(q[sl])
        mp['qkT'] = _featmajor(qk[sl])
        mp['peT'] = _featmajor(pe[sl])
        mp['mT'] = _featmajor(m[sl])
        in_maps.append(mp)
    return in_maps


def kernel(**inputs):
    B, NQ, W = inputs['query'].shape
    S = inputs['enc_mem'].shape[1]
    MLP = inputs['ffn_w1'].shape[0]
    NH = 16
    assert B % N_CORES == 0
    B_loc = B // N_CORES

    nc = _get_program(B_loc, NQ, S, W, NH, MLP)
    in_maps = _make_in_maps(inputs)

    res = run_bass_kernel_spmd(nc, in_maps, list(range(N_CORES)))
    return np.concatenate(
        [np.asarray(res.results[c]["out"]).astype(np.float32)
         for c in range(N_CORES)], axis=0)
